# revision 1
# baseline (speedup 1.0000x reference)
"""FFM layer kernel for Trainium2 (8 NeuronCores, SPMD batch-parallel).

out = b + x @ W + 0.5 * (x^T A x - sum_i x_i^2 A_ii),
A[i,j] = <V[i, field(j)], V[j, field(i)]>.

Strategy: columns of x are sorted by field on the host so each field group
occupies a contiguous, 64-aligned partition range.  Per batch-tile of 128
samples the tensor engine computes the field-factorized tensor
    T[b, c1, (c2,f)] = sum_{i in group c1} x[b,i] * V[i,c2,f]
(one matmul per group, x stationary, [V|W] streaming) and a d-weighted gram
whose diagonal yields sum_i d_i x_i^2 (d_i = |V[i, field(i)]|^2).  The
quadratic form is then
    x^T A x = sum_{c1,c2,f} T[b,c1,c2,f] * T[b,c2,c1,f],
reduced on the vector engine with fused tensor_tensor_reduce over the strict
lower triangle (weight 2 by symmetry) plus the block diagonal, while the
scalar engine copies finished T blocks from PSUM to SBUF.
"""

import sys

for _p in ("/opt/trn_rl_repo",):
    if _p not in sys.path:
        sys.path.insert(0, _p)

import numpy as np

import concourse.bass as bass
import concourse.tile as tile
from concourse import bacc, bass_utils, mybir

F32 = mybir.dt.float32
F16 = mybir.dt.float16

B, D, FIELDS, F = 4096, 2000, 40, 8
NCORES = 8
BS = B // NCORES          # batch shard per core (512)
BT = BS // 128            # batch tiles per core (4)
CF = FIELDS * F           # 320
VW = CF + 1               # V block + W column
MERGE = 2                 # PSUM blocks per merged copy / psum tile


def _placement(counts):
    """Assign each field group a start row; groups <=64 rows go in 64-row
    slots, bigger groups take a whole 128-row tile alone."""
    offs = [0] * FIELDS
    pos = 0
    for c in range(FIELDS):
        n = int(counts[c])
        if n == 0:
            offs[c] = pos
            continue
        if n <= 64:
            if pos % 64 != 0:
                pos = (pos // 64 + 1) * 64
            # a 64-slot cannot host >64 rows; <=64 fits by construction
        else:
            if pos % 128 != 0:
                pos = (pos // 128 + 1) * 128
            assert n <= 128, f"field group of {n} > 128 rows unsupported"
        offs[c] = pos
        pos += n
    dp = ((pos + 127) // 128) * 128
    return offs, dp


def _ap3(sliced, d1, s1, d2, s2):
    """Re-dim a 2-D [part, free] AP into [part, d1(stride s1), d2(stride s2)]."""
    p = sliced.ap[0]
    return bass.AP(
        tensor=sliced.tensor,
        offset=sliced.offset,
        ap=[[p[0], p[1]], [s1, d1], [s2, d2]],
    )


def _build(groups, ntiles, dp):
    """Build + compile the per-core program.  groups: list of (c, off, n)."""
    nc = bacc.Bacc(
        "TRN2",
        target_bir_lowering=False,
        debug=False,
        enable_asserts=False,
        num_devices=NCORES,
    )
    xt_d = nc.dram_tensor("xt", [128, ntiles * 512], F16, kind="ExternalInput").ap()
    vrw_d = nc.dram_tensor("vrw", [128, ntiles * VW], F16, kind="ExternalInput").ap()
    dxh_d = nc.dram_tensor("dxh", [128, ntiles * 512], F16, kind="ExternalInput").ap()
    brep_d = nc.dram_tensor("brep", [128, 1], F32, kind="ExternalInput").ap()
    ident_d = nc.dram_tensor("ident", [128, 128], F32, kind="ExternalInput").ap()
    out_d = nc.dram_tensor("out", [BS, 1], F32, kind="ExternalOutput").ap()

    by_tile = [[] for _ in range(ntiles)]
    for c, off, n in groups:
        if n > 0:
            by_tile[off // 128].append((c, off, n))

    with tile.TileContext(nc) as tc:
        with (
            tc.tile_pool(name="big", bufs=1) as big,
            tc.tile_pool(name="small", bufs=1) as small,
            tc.tile_pool(name="parts", bufs=2) as parts_pool,
            tc.tile_pool(name="tsbp", bufs=2) as tsb_pool,
            tc.tile_pool(name="scratch", bufs=2) as scratch_pool,
            tc.tile_pool(name="outp", bufs=2) as out_pool,
            tc.tile_pool(name="qp", bufs=3, space="PSUM") as qpool,
            tc.tile_pool(name="gramp", bufs=2, space="PSUM") as gram_pool,
        ):
            xs = big.tile([128, ntiles * 512], F16, tag="xs")
            vrw = big.tile([128, ntiles * VW], F16, tag="vrw")
            dx = big.tile([128, ntiles * 512], F16, tag="dx")
            brep = small.tile([128, 1], F32)
            ident = small.tile([128, 128], F32)

            nc.scalar.dma_start(brep[:], brep_d[:, :])
            nc.scalar.dma_start(ident[:], ident_d[:, :])
            # chunked loads (host packs SBUF layout; plain contiguous 2-D DMAs)
            bounds = [0, 2]
            while bounds[-1] < ntiles:
                bounds.append(min(bounds[-1] + 5, ntiles))
            for t0, t1 in zip(bounds, bounds[1:]):
                nc.scalar.dma_start(
                    vrw[:, t0 * VW : t1 * VW], vrw_d[:, t0 * VW : t1 * VW]
                )
                nc.sync.dma_start(
                    xs[:, t0 * 512 : t1 * 512], xt_d[:, t0 * 512 : t1 * 512]
                )
                nc.sync.dma_start(
                    dx[:, t0 * 512 : t1 * 512], dxh_d[:, t0 * 512 : t1 * 512]
                )

            ginfo = {c: (off, n) for c, off, n in groups}
            # tile t is complete once the merge-group containing its last field ends
            last_c_of_tile = {}
            for c, off, n in groups:
                if n > 0:
                    t = off // 128
                    last_c_of_tile[t] = max(last_c_of_tile.get(t, -1), c)
            NQ = (FIELDS + MERGE - 1) // MERGE
            tiles_done_at_q = [[] for _ in range(NQ)]
            for t, lc in last_c_of_tile.items():
                tiles_done_at_q[lc // MERGE].append(t)
            for bt in range(BT):
                partials = parts_pool.tile([128, FIELDS + 2], F32, tag="partials")
                gram = gram_pool.tile([128, 128], F32, tag="gram")
                tsb = tsb_pool.tile([128, FIELDS * VW], F16, tag="tsb")
                tsb3 = tsb[:].rearrange("p (c s) -> p c s", s=VW)
                for q in range(NQ):
                    c0 = q * MERGE
                    cs = list(range(c0, min(c0 + MERGE, FIELDS)))
                    qt = qpool.tile([128, MERGE * 512], F32, tag="qt")
                    for c in cs:
                        off, n = ginfo[c]
                        slot = c - c0
                        if n == 0:
                            nc.vector.memset(
                                qt[:, slot * 512 : slot * 512 + VW], 0.0
                            )
                            continue
                        t = off // 128
                        lp = off % 128
                        if n <= 64:
                            base, kk = (lp // 64) * 64, 64
                        else:
                            base, kk = 0, 128
                        xcol = t * 512 + bt * 128
                        nc.tensor.matmul(
                            qt[:, slot * 512 : slot * 512 + VW],
                            xs[base : base + kk, xcol : xcol + 128],
                            vrw[base : base + kk, t * VW : t * VW + VW],
                            start=True,
                            stop=True,
                        )
                    # reduce for the first block of the group only needs
                    # earlier groups' copies -- emit it before this group's copy
                    def _stt(c, slot):
                        if c == 0:
                            nc.vector.memset(partials[:, 0:1], 0.0)
                            return
                        sc = scratch_pool.tile([128, 512], F32, tag="sc", name=f"sc{c}")
                        in0 = _ap3(
                            qt[:, slot * 512 : slot * 512 + 8 * c], c, 8, 8, 1
                        )
                        in1 = tsb3[:, 0:c, 8 * c : 8 * c + 8]
                        nc.vector.scalar_tensor_tensor(
                            _ap3(sc[:, : 8 * c], c, 8, 8, 1),
                            in0,
                            1.0,
                            in1,
                            op0=mybir.AluOpType.mult,
                            op1=mybir.AluOpType.mult,
                            accum_out=partials[:, c : c + 1],
                        )

                    # copy finished merge-group of blocks to SBUF (trimmed)
                    trim = 8 * c0
                    src_ap = qt[:].rearrange("p (g e) -> p g e", e=512)[
                        :, : len(cs), trim:VW
                    ]
                    dst = tsb3[:, c0 : c0 + len(cs), trim:VW]
                    nc.scalar.copy(dst, src_ap)
                    for c in cs:
                        _stt(c, c - c0)
                    # d-weighted gram accumulates over tiles as they finish
                    for t in sorted(tiles_done_at_q[q]):
                        xcol = t * 512 + bt * 128
                        nc.tensor.matmul(
                            gram[:],
                            xs[:, xcol : xcol + 128],
                            dx[:, xcol : xcol + 128],
                            start=(t == min(last_c_of_tile)),
                            stop=(t == max(last_c_of_tile)),
                        )
                # block-diagonal term: sum_f T[c,c,f]^2, weight 0.5
                tap = tsb[:]
                diag_ap = bass.AP(
                    tensor=tap.tensor,
                    offset=tap.offset,
                    ap=[[tap.ap[0][0], tap.ap[0][1]], [VW + 8, FIELDS], [1, F]],
                )
                scd = scratch_pool.tile([128, 512], F32, tag="sc")
                nc.vector.scalar_tensor_tensor(
                    _ap3(scd[:, : FIELDS * F], FIELDS, 8, 8, 1),
                    diag_ap,
                    0.5,
                    diag_ap,
                    op0=mybir.AluOpType.mult,
                    op1=mybir.AluOpType.mult,
                    accum_out=partials[:, FIELDS : FIELDS + 1],
                )
                # gram diagonal: subtract 0.5 * sum_i d_i x_i^2
                scg = scratch_pool.tile([128, 512], F32, tag="sc")
                nc.vector.scalar_tensor_tensor(
                    scg[:, :128],
                    gram[:],
                    -0.5,
                    ident[:],
                    op0=mybir.AluOpType.mult,
                    op1=mybir.AluOpType.mult,
                    accum_out=partials[:, FIELDS + 1 : FIELDS + 2],
                )
                # W column (tsb3[:, :, CF]) summed over blocks, plus all partials
                wsum = out_pool.tile([128, 1], F32, tag="wsum")
                nc.vector.tensor_reduce(
                    wsum[:],
                    tsb3[:, :, CF : CF + 1],
                    axis=mybir.AxisListType.XY,
                    op=mybir.AluOpType.add,
                )
                psum_red = out_pool.tile([128, 1], F32, tag="psum_red")
                nc.vector.tensor_reduce(
                    psum_red[:],
                    partials[:],
                    axis=mybir.AxisListType.X,
                    op=mybir.AluOpType.add,
                )
                ob = out_pool.tile([128, 1], F32, tag="ob")
                nc.vector.tensor_tensor(
                    ob[:], psum_red[:], wsum[:], op=mybir.AluOpType.add
                )
                ob2 = out_pool.tile([128, 1], F32, tag="ob2")
                nc.vector.tensor_scalar(
                    ob2[:],
                    ob[:],
                    brep[:, 0:1],
                    None,
                    op0=mybir.AluOpType.add,
                )
                nc.sync.dma_start(out_d[bt * 128 : (bt + 1) * 128, :], ob2[:])

    nc.compile()
    return nc


def _host_prep(x, field_dict, b, W, V):
    x = np.ascontiguousarray(np.asarray(x, np.float32))
    fd = np.asarray(field_dict).astype(np.int64)
    W = np.asarray(W, np.float32)
    V = np.asarray(V, np.float32)
    b = np.asarray(b, np.float32)

    perm = np.argsort(fd, kind="stable")
    counts = np.bincount(fd[perm], minlength=FIELDS)
    offs, dp = _placement(counts)
    ntiles = dp // 128

    xt = np.zeros((dp, B), np.float32)
    vrw = np.zeros((dp, VW), np.float32)
    dpad = np.zeros((dp,), np.float32)
    groups = []
    src = 0
    for c in range(FIELDS):
        n = int(counts[c])
        o = offs[c]
        groups.append((c, o, n))
        if n:
            idx = perm[src : src + n]
            xt[o : o + n, :] = x[:, idx].T
            vrw[o : o + n, :CF] = V[idx].reshape(n, CF)
            vrw[o : o + n, CF] = W[idx, 0]
            dpad[o : o + n] = (V[idx, fd[idx], :] ** 2).sum(-1)
            src += n
    dxh = (dpad[:, None] * xt).astype(np.float16)
    xt = xt.astype(np.float16)
    vrw = vrw.astype(np.float16)
    # pack to SBUF layout [128, ntiles * cols] (partition-major)
    xt = xt.reshape(ntiles, 128, B).transpose(1, 0, 2)
    dxh = dxh.reshape(ntiles, 128, B).transpose(1, 0, 2)
    vrw = np.ascontiguousarray(
        vrw.reshape(ntiles, 128, VW).transpose(1, 0, 2)
    ).reshape(128, ntiles * VW)
    brep = np.full((128, 1), float(b[0]), np.float32)
    ident = np.eye(128, dtype=np.float32)
    return xt, vrw, dxh, brep, ident, groups, ntiles, dp


def kernel(x, field_dict, b, W, V):
    xt, vrw, dxh, brep, ident, groups, ntiles, dp = _host_prep(
        x, field_dict, b, W, V
    )
    nc = _build(groups, ntiles, dp)
    in_maps = []
    for core in range(NCORES):
        in_maps.append(
            {
                "xt": np.ascontiguousarray(
                    xt[:, :, core * BS : (core + 1) * BS]
                ).reshape(128, -1),
                "dxh": np.ascontiguousarray(
                    dxh[:, :, core * BS : (core + 1) * BS]
                ).reshape(128, -1),
                "vrw": vrw,
                "brep": brep,
                "ident": ident,
            }
        )
    res = bass_utils.run_bass_kernel_spmd(
        nc, in_maps, core_ids=list(range(NCORES))
    )
    out = np.concatenate([r["out"] for r in res.results], axis=0)
    return out.astype(np.float32)


if __name__ == "__main__":
    rng = np.random.default_rng(0)
    x = rng.standard_normal((B, D), dtype=np.float32)
    fd = rng.integers(0, FIELDS, size=(D,)).astype(np.int32)
    b = np.zeros((1,), np.float32)
    W = (rng.standard_normal((D, 1)) * 0.01).astype(np.float32)
    V = (rng.standard_normal((D, FIELDS, F)) * 0.01).astype(np.float32)
    out = kernel(x=x, field_dict=fd, b=b, W=W, V=V)
    print(out.shape, out.dtype, out[:4, 0])

